# revision 5
# baseline (speedup 1.0000x reference)
"""Multi-head self-attention + projector, Trainium2 Bass kernel, 8 NeuronCores.

Reference computation (per batch b):
    Q = X @ Wq + bq; K = X @ Wk + bk; V = X @ Wv + bv      (X: [S, D])
    per head h: P_h = softmax(Q_h K_h^T / sqrt(dh)); A_h = P_h V_h
    Y = concat_h(A_h) @ Wo + bo

Sharding: core i handles batch i//2, query rows (i%2)*1024 .. +1024.
K/V are computed for the full sequence on each core (no collectives).
The host rolls each core's query columns to the front of X^T so a single
SPMD program serves all 8 cores.

Algebraic simplifications (all exact w.r.t. softmax):
  - bk dropped: (Q+bq)(K0+bk)^T/8 differs from (Q+bq)K0^T/8 by a per-query
    constant, which softmax cancels.
  - bv folded into the output bias on host: P @ (V0 + 1 bv^T) @ Wo + bo =
    (P @ V0) @ Wo + (bv @ Wo + bo) since softmax rows sum to 1.
  - no max-subtraction in softmax: scores are O(1) for these inputs.

Device pipeline per core (all matmuls fp32r except attended, which is bf16):
  phase A: Q^T[hid,1024] (+bq), K^T[hid,2048], V[2048, hid] (bf16, with a
           per-head ones column for free softmax row sums)
  phase B: per (q-block 512, head-pair): stream k in 128-chunks:
           scoresT[k,q] via PE (head pair packed in rows 0:64/64:128 of the
           array), exp on ACT (bf16 out), attended^T accumulation on PE
           (lhsT = [V_h | ones]).  Normalize by the row-sum row, write
           attended^T[hid, q]; odd heads shifted to partitions 64:128 via a
           small SBUF->SBUF DMA.
  phase C: Y[q,768] = attended^T.T @ Wo + bo' per 128-row q-tile.
"""

import numpy as np

import concourse.bass as bass
import concourse.mybir as mybir
import concourse.tile as tile
from concourse import bacc, bass_utils

F32 = mybir.dt.float32
F32R = mybir.dt.float32r
BF16 = mybir.dt.bfloat16

B, S, D, HID, HEADS, DH, VD = 4, 2048, 768, 512, 8, 64, 768
N_CORES = 8
SQ = S // 2  # query rows per core
DC = D // 128  # 6 contraction chunks for the projections
HC = HID // 128  # 4 hidden chunks
KT = S // 128  # 16 key chunks
QB = SQ // 512  # 2 query blocks of 512


def _kernel_body(tc):
    nc = tc.nc
    xt_d = nc.dram_tensor("xt", [D, S], F32R, kind="ExternalInput").ap()
    wq_d = nc.dram_tensor("wq", [D, HID], F32R, kind="ExternalInput").ap()
    wk_d = nc.dram_tensor("wk", [D, HID], F32R, kind="ExternalInput").ap()
    wv_d = nc.dram_tensor("wv", [D, HID], F32R, kind="ExternalInput").ap()
    bq_d = nc.dram_tensor("bq", [HID], F32, kind="ExternalInput").ap()
    wo_d = nc.dram_tensor("wo", [HID, VD], F32R, kind="ExternalInput").ap()
    bo_d = nc.dram_tensor("bo2", [VD], F32, kind="ExternalInput").ap()
    y_d = nc.dram_tensor("y", [SQ, VD], F32, kind="ExternalOutput").ap()

    ctx = tc  # alias

    with (
        tc.tile_pool(name="persist", bufs=1) as persist,
        tc.tile_pool(name="scores_ps", bufs=2, space="PSUM") as scores_ps_pool,
        tc.tile_pool(name="att_ps", bufs=1, space="PSUM") as att_ps_pool,
    ):
        # ---- persistent SBUF tensors ----
        wo_sb = persist.tile([128, HC, VD], F32R)
        bo_sb = persist.tile([128, VD], F32)
        bq_sb = persist.tile([128, HC], F32)
        qt_sb = persist.tile([128, HC, SQ], F32R)
        kt_sb = persist.tile([128, HC, S], F32R)
        # V in [seq, head, 65]: even head -> [V(64) | ones], odd -> same
        v_sb = persist.tile([128, KT, HEADS, DH + 1], BF16)
        att_sb = persist.tile([128, HC, SQ], F32R)
        zero_sb = persist.tile([128, 1], F32)

        nc.vector.memset(zero_sb[:], 0.0)
        nc.vector.memset(v_sb[:, :, :, DH : DH + 1], 1.0)

        for c in range(HC):
            nc.sync.dma_start(
                out=wo_sb[:, c, :],
                in_=wo_d.rearrange("(c p) v -> c p v", c=HC)[c],
            )
        # broadcast bo' across partitions with a stride-0 DRAM AP
        nc.sync.dma_start(
            out=bo_sb[:],
            in_=bass.AP(tensor=bo_d.tensor, offset=bo_d.offset, ap=[[0, 128], [1, VD]]),
        )
        nc.sync.dma_start(out=bq_sb[:], in_=bq_d.rearrange("(c p) -> p c", c=HC))

        # ---- phase A: projections ----
        with (
            tc.tile_pool(name="pa_psum", bufs=2, space="PSUM") as pa_psum,
            tc.tile_pool(name="pa_sbuf", bufs=1) as pa_sbuf,
        ):
            xt_sb = pa_sbuf.tile([128, DC, S], F32R)
            wq_sb = pa_sbuf.tile([128, DC, HID], F32R)
            wk_sb = pa_sbuf.tile([128, DC, HID], F32R)
            wv_sb = pa_sbuf.tile([128, DC, HID], F32R)

            xt_r = xt_d.rearrange("(c p) s -> c p s", c=DC)
            for c in range(DC):
                nc.sync.dma_start(out=xt_sb[:, c, :], in_=xt_r[c])
            for w_sb, w_d in ((wq_sb, wq_d), (wk_sb, wk_d), (wv_sb, wv_d)):
                w_r = w_d.rearrange("(c p) h -> c p h", c=DC)
                for c in range(DC):
                    nc.sync.dma_start(out=w_sb[:, c, :], in_=w_r[c])

            # Q^T [hid, q] with +bq, K^T [hid, k] (no bias)
            for c in range(HC):
                for qb in range(QB):
                    ps = pa_psum.tile([128, 512], F32, tag="pa")
                    for d in range(DC):
                        nc.tensor.matmul(
                            ps[:],
                            wq_sb[:, d, c * 128 : (c + 1) * 128],
                            xt_sb[:, d, qb * 512 : (qb + 1) * 512],
                            start=(d == 0),
                            stop=(d == DC - 1),
                        )
                    nc.scalar.activation(
                        out=qt_sb[:, c, qb * 512 : (qb + 1) * 512],
                        in_=ps[:],
                        func=mybir.ActivationFunctionType.Identity,
                        bias=bq_sb[:, c : c + 1],
                        scale=1.0,
                    )
                for sb in range(S // 512):
                    ps = pa_psum.tile([128, 512], F32, tag="pa")
                    for d in range(DC):
                        nc.tensor.matmul(
                            ps[:],
                            wk_sb[:, d, c * 128 : (c + 1) * 128],
                            xt_sb[:, d, sb * 512 : (sb + 1) * 512],
                            start=(d == 0),
                            stop=(d == DC - 1),
                        )
                    nc.scalar.activation(
                        out=kt_sb[:, c, sb * 512 : (sb + 1) * 512],
                        in_=ps[:],
                        func=mybir.ActivationFunctionType.Identity,
                        bias=zero_sb[:, 0:1],
                        scale=1.0,
                    )

            # V [seq, hid] in bf16, interleaved with per-head ones columns
            for st in range(KT):
                ps = pa_psum.tile([128, 512], F32, tag="pa")
                for d in range(DC):
                    nc.tensor.matmul(
                        ps[:],
                        xt_sb[:, d, st * 128 : (st + 1) * 128],
                        wv_sb[:, d, :],
                        start=(d == 0),
                        stop=(d == DC - 1),
                    )
                ps_h = ps[:].rearrange("p (h d) -> p h d", h=HEADS)
                nc.vector.tensor_copy(
                    out=v_sb[:, st, :, 0:DH],
                    in_=ps_h,
                )

        # ---- phase B: attention ----
        with (
            tc.tile_pool(name="e_pool", bufs=6) as e_pool,
            tc.tile_pool(name="rb_pool", bufs=2) as rb_pool,
            tc.tile_pool(name="tmp_pool", bufs=2) as tmp_pool,
            tc.tile_pool(name="y_ps", bufs=1, space="PSUM") as y_ps_pool,
            tc.tile_pool(name="y_sb", bufs=2) as y_sb_pool,
        ):
            for qb in range(QB):
                for hp in range(HEADS // 2):
                    h0, h1 = 2 * hp, 2 * hp + 1
                    att0 = att_ps_pool.tile([128, 512], F32, tag="att0")
                    att1 = att_ps_pool.tile([128, 512], F32, tag="att1")
                    for kt in range(KT):
                        s_ps = scores_ps_pool.tile([128, 2, 512], F32, tag="s")
                        qs = qt_sb[:, hp, qb * 512 : (qb + 1) * 512]
                        ks = kt_sb[:, hp, kt * 128 : (kt + 1) * 128]
                        nc.tensor.matmul(
                            s_ps[:, 0, :],
                            ks[0:64, :],
                            qs[0:64, :],
                            start=True,
                            stop=True,
                        )
                        nc.tensor.matmul(
                            s_ps[:, 1, :],
                            ks[64:128, :],
                            qs[64:128, :],
                            start=True,
                            stop=True,
                        )
                        e = e_pool.tile([128, 2, 512], BF16, tag="e")
                        nc.scalar.activation(
                            out=e[:],
                            in_=s_ps[:],
                            func=mybir.ActivationFunctionType.Exp,
                            bias=zero_sb[:, 0:1],
                            scale=0.125,
                        )
                        nc.tensor.matmul(
                            att0[0 : DH + 1, :],
                            v_sb[:, kt, h0, :],
                            e[:, 0, :],
                            start=(kt == 0),
                            stop=(kt == KT - 1),
                        )
                        nc.tensor.matmul(
                            att1[0 : DH + 1, :],
                            v_sb[:, kt, h1, :],
                            e[:, 1, :],
                            start=(kt == 0),
                            stop=(kt == KT - 1),
                        )
                    # normalize: rows 0:64 scaled by 1/row 64
                    for h, att in ((h0, att0), (h1, att1)):
                        rec = rb_pool.tile([DH + 1, 512], F32, tag="rec")
                        nc.vector.reciprocal(rec[DH : DH + 1, :], att[DH : DH + 1, :])
                        rec0 = rb_pool.tile([1, 512], F32, tag="rec0")
                        nc.sync.dma_start(rec0[0:1, :], rec[DH : DH + 1, :])
                        rb = rb_pool.tile([64, 512], F32, tag="rb")
                        nc.gpsimd.partition_broadcast(rb[:], rec0[0:1, :])
                        dst_cols = att_sb[:, hp, qb * 512 : (qb + 1) * 512]
                        if h % 2 == 0:
                            nc.vector.tensor_mul(
                                dst_cols[0:64, :], att[0:DH, :], rb[:]
                            )
                        else:
                            tmp_n = tmp_pool.tile([64, 512], F32R, tag="tmp")
                            nc.vector.tensor_mul(tmp_n[:], att[0:DH, :], rb[:])
                            nc.sync.dma_start(out=dst_cols[64:128, :], in_=tmp_n[:])

                # ---- phase C for this q-block ----
                for t in range(4):
                    qt_i = qb * 4 + t
                    y_ps = y_ps_pool.tile([128, VD], F32, tag="y")
                    for c in range(HC):
                        lhsT = att_sb[:, c, qt_i * 128 : (qt_i + 1) * 128]
                        nc.tensor.matmul(
                            y_ps[:, 0:512],
                            lhsT,
                            wo_sb[:, c, 0:512],
                            start=(c == 0),
                            stop=(c == HC - 1),
                        )
                        nc.tensor.matmul(
                            y_ps[:, 512:VD],
                            lhsT,
                            wo_sb[:, c, 512:VD],
                            start=(c == 0),
                            stop=(c == HC - 1),
                        )
                    y_sb = y_sb_pool.tile([128, VD], F32, tag="ysb")
                    nc.vector.tensor_add(y_sb[:, 0:512], y_ps[:, 0:512], bo_sb[:, 0:512])
                    nc.vector.tensor_add(y_sb[:, 512:VD], y_ps[:, 512:VD], bo_sb[:, 512:VD])
                    nc.sync.dma_start(
                        out=y_d.rearrange("(t p) v -> t p v", p=128)[qt_i],
                        in_=y_sb[:],
                    )


_BUILT = None


def _build():
    global _BUILT
    if _BUILT is None:
        nc = bacc.Bacc(
            "TRN2", target_bir_lowering=False, debug=False, num_devices=N_CORES
        )
        with tile.TileContext(nc) as tc:
            _kernel_body(tc)
        nc.compile()
        _BUILT = nc
    return _BUILT


def _prepare_in_maps(text_embeds, Wq, bq, Wk, bk, Wv, bv, Wo, bo):
    text_embeds = np.asarray(text_embeds, np.float32)
    Wq = np.ascontiguousarray(np.asarray(Wq, np.float32))
    Wk = np.ascontiguousarray(np.asarray(Wk, np.float32))
    Wv = np.ascontiguousarray(np.asarray(Wv, np.float32))
    Wo = np.ascontiguousarray(np.asarray(Wo, np.float32))
    bq = np.ascontiguousarray(np.asarray(bq, np.float32))
    bo2 = (
        np.asarray(bo, np.float64) + np.asarray(bv, np.float64) @ np.asarray(Wo, np.float64)
    ).astype(np.float32)
    in_maps = []
    for core in range(N_CORES):
        b, half = divmod(core, 2)
        xt = np.ascontiguousarray(text_embeds[b].T)  # [D, S]
        if half:
            xt = np.ascontiguousarray(np.roll(xt, -SQ, axis=1))
        in_maps.append(
            {
                "xt": xt,
                "wq": Wq,
                "wk": Wk,
                "wv": Wv,
                "bq": bq,
                "wo": Wo,
                "bo2": bo2,
            }
        )
    return in_maps


def _assemble(results):
    out = np.empty((B, S, VD), np.float32)
    for core in range(N_CORES):
        b, half = divmod(core, 2)
        out[b, half * SQ : (half + 1) * SQ] = results[core]["y"]
    return out


def run(trace=False, **inputs):
    nc = _build()
    in_maps = _prepare_in_maps(**inputs)
    res = bass_utils.run_bass_kernel_spmd(
        nc, in_maps, core_ids=list(range(N_CORES)), trace=trace
    )
    return _assemble(res.results), res


def kernel(**inputs):
    out, _ = run(trace=False, **inputs)
    return out


# revision 7
# speedup vs baseline: 1.1218x; 1.1218x over previous
"""Multi-head self-attention + projector, Trainium2 Bass kernel, 8 NeuronCores.

Reference computation (per batch b):
    Q = X @ Wq + bq; K = X @ Wk + bk; V = X @ Wv + bv      (X: [S, D])
    per head h: P_h = softmax(Q_h K_h^T / sqrt(dh)); A_h = P_h V_h
    Y = concat_h(A_h) @ Wo + bo

Sharding: core i handles batch i//2, query rows (i%2)*1024 .. +1024.
K/V are computed for the full sequence on each core (no collectives).
The host rolls each core's query columns to the front of X^T so a single
SPMD program serves all 8 cores.

Algebraic simplifications (all exact w.r.t. softmax):
  - bk dropped: (Q+bq)(K0+bk)^T/8 differs from (Q+bq)K0^T/8 by a per-query
    constant, which softmax cancels.
  - bv folded into the output bias on host: P @ (V0 + 1 bv^T) @ Wo + bo =
    (P @ V0) @ Wo + (bv @ Wo + bo) since softmax rows sum to 1.
  - no max-subtraction in softmax: scores are O(1) for these inputs.

Device pipeline per core (all matmuls fp32r except attended, which is bf16):
  phase A: Q^T[hid,1024] (+bq), K^T[hid,2048], V[2048, hid] (bf16, with a
           per-head ones column for free softmax row sums)
  phase B: per (q-block 512, head-pair): stream k in 128-chunks:
           scoresT[k,q] via PE (head pair packed in rows 0:64/64:128 of the
           array), exp on ACT (bf16 out), attended^T accumulation on PE
           (lhsT = [V_h | ones]).  Normalize by the row-sum row, write
           attended^T[hid, q]; odd heads shifted to partitions 64:128 via a
           small SBUF->SBUF DMA.
  phase C: Y[q,768] = attended^T.T @ Wo + bo' per 128-row q-tile.
"""

import numpy as np

import concourse.bass as bass
import concourse.mybir as mybir
import concourse.tile as tile
from concourse import bacc, bass_utils

F32 = mybir.dt.float32
F32R = mybir.dt.float32r
BF16 = mybir.dt.bfloat16

B, S, D, HID, HEADS, DH, VD = 4, 2048, 768, 512, 8, 64, 768
N_CORES = 8
SQ = S // 2  # query rows per core
DC = D // 128  # 6 contraction chunks for the projections
HC = HID // 128  # 4 hidden chunks
KT = S // 128  # 16 key chunks
QB = SQ // 512  # 2 query blocks of 512


def _kernel_body(tc):
    nc = tc.nc
    xt_d = nc.dram_tensor("xt", [D, S], F32R, kind="ExternalInput").ap()
    wq_d = nc.dram_tensor("wq", [D, HID], F32R, kind="ExternalInput").ap()
    wk_d = nc.dram_tensor("wk", [D, HID], F32R, kind="ExternalInput").ap()
    wv_d = nc.dram_tensor("wv", [D, HID], F32R, kind="ExternalInput").ap()
    bq_d = nc.dram_tensor("bq", [HID], F32, kind="ExternalInput").ap()
    wo_d = nc.dram_tensor("wo", [HID, VD], F32R, kind="ExternalInput").ap()
    bo_d = nc.dram_tensor("bo2", [VD], F32, kind="ExternalInput").ap()
    y_d = nc.dram_tensor("y", [SQ, VD], F32, kind="ExternalOutput").ap()

    ctx = tc  # alias

    with (
        tc.tile_pool(name="persist", bufs=1) as persist,
        tc.tile_pool(name="mm_ps", bufs=3, space="PSUM") as mm_ps_pool,
        tc.tile_pool(name="att_ps", bufs=1, space="PSUM") as att_ps_pool,
    ):
        # ---- persistent SBUF tensors ----
        wo_sb = persist.tile([128, HC, VD], F32R)
        bo_sb = persist.tile([128, VD], F32)
        bq_sb = persist.tile([128, HC], F32)
        qt_sb = persist.tile([128, HC, SQ], F32R)
        kt_sb = persist.tile([128, HC, S], F32R)
        # V in [seq, head, 65]: even head -> [V(64) | ones], odd -> same
        v_sb = persist.tile([128, KT, HEADS, DH + 1], BF16)
        att_sb = persist.tile([128, HC, SQ], F32R)
        zero_sb = persist.tile([128, 1], F32)

        nc.vector.memset(zero_sb[:], 0.0)
        nc.vector.memset(v_sb[:, :, :, DH : DH + 1], 1.0)

        for c in range(HC):
            nc.sync.dma_start(
                out=wo_sb[:, c, :],
                in_=wo_d.rearrange("(c p) v -> c p v", c=HC)[c],
            )
        # broadcast bo' across partitions with a stride-0 DRAM AP
        nc.sync.dma_start(
            out=bo_sb[:],
            in_=bass.AP(tensor=bo_d.tensor, offset=bo_d.offset, ap=[[0, 128], [1, VD]]),
        )
        nc.sync.dma_start(out=bq_sb[:], in_=bq_d.rearrange("(c p) -> p c", c=HC))

        # ---- phase A: projections ----
        with tc.tile_pool(name="pa_sbuf", bufs=1) as pa_sbuf:
            xt_sb = pa_sbuf.tile([128, DC, S], F32R)
            wq_sb = pa_sbuf.tile([128, DC, HID], F32R)
            wk_sb = pa_sbuf.tile([128, DC, HID], F32R)
            wv_sb = pa_sbuf.tile([128, DC, HID], F32R)

            # interleave input DMAs chunk-by-chunk so compute can chase arrivals
            xt_r = xt_d.rearrange("(c p) s -> c p s", c=DC)
            for c in range(DC):
                nc.sync.dma_start(out=xt_sb[:, c, :], in_=xt_r[c])
                for w_sb, w_d in ((wq_sb, wq_d), (wk_sb, wk_d), (wv_sb, wv_d)):
                    w_r = w_d.rearrange("(c p) h -> c p h", c=DC)
                    nc.sync.dma_start(out=w_sb[:, c, :], in_=w_r[c])

            # accumulation jobs: (lhsT source, rhs source, epilogue)
            jobs = []
            for c in range(HC):
                for qb in range(QB):
                    jobs.append(("q", c, qb))
                for sb in range(S // 512):
                    jobs.append(("k", c, sb))
            for st in range(KT):
                jobs.append(("v", st, 0))

            def emit_job(kind, a, b, ps):
                # staggered contraction order so early chunks start early
                for i in range(DC):
                    d = (emit_job.off + i) % DC
                    if kind == "q":
                        lhsT = wq_sb[:, d, a * 128 : (a + 1) * 128]
                        rhs = xt_sb[:, d, b * 512 : (b + 1) * 512]
                    elif kind == "k":
                        lhsT = wk_sb[:, d, a * 128 : (a + 1) * 128]
                        rhs = xt_sb[:, d, b * 512 : (b + 1) * 512]
                    else:
                        lhsT = xt_sb[:, d, a * 128 : (a + 1) * 128]
                        rhs = wv_sb[:, d, :]
                    nc.tensor.matmul(
                        ps, lhsT, rhs, start=(i == 0), stop=(i == DC - 1)
                    )
                if kind == "q":
                    nc.scalar.activation(
                        out=qt_sb[:, a, b * 512 : (b + 1) * 512],
                        in_=ps,
                        func=mybir.ActivationFunctionType.Identity,
                        bias=bq_sb[:, a : a + 1],
                        scale=1.0,
                    )
                elif kind == "k":
                    nc.scalar.activation(
                        out=kt_sb[:, a, b * 512 : (b + 1) * 512],
                        in_=ps,
                        func=mybir.ActivationFunctionType.Identity,
                        bias=zero_sb[:, 0:1],
                        scale=1.0,
                    )
                else:
                    nc.vector.tensor_copy(
                        out=v_sb[:, a, :, 0:DH],
                        in_=ps.rearrange("p (h d) -> p h d", h=HEADS),
                    )

            emit_job.off = 0
            for j in range(0, len(jobs), 2):
                ps2 = mm_ps_pool.tile([128, 2, 512], F32, tag="mm")
                for s_i, job in enumerate(jobs[j : j + 2]):
                    emit_job.off = (j + s_i) % DC
                    emit_job(*job, ps2[:, s_i, :])

        # ---- phase B: attention ----
        with (
            tc.tile_pool(name="e_pool", bufs=6) as e_pool,
            tc.tile_pool(name="rb_pool", bufs=2) as rb_pool,
            tc.tile_pool(name="tmp_pool", bufs=2) as tmp_pool,
            tc.tile_pool(name="y_sb", bufs=2) as y_sb_pool,
        ):
            for qb in range(QB):
                for hp in range(HEADS // 2):
                    h0, h1 = 2 * hp, 2 * hp + 1
                    att0 = att_ps_pool.tile([128, 512], F32, tag="att0")
                    att1 = att_ps_pool.tile([128, 512], F32, tag="att1")
                    for kt in range(KT):
                        s_ps = mm_ps_pool.tile([128, 2, 512], F32, tag="mm")
                        qs = qt_sb[:, hp, qb * 512 : (qb + 1) * 512]
                        ks = kt_sb[:, hp, kt * 128 : (kt + 1) * 128]
                        nc.tensor.matmul(
                            s_ps[:, 0, :],
                            ks[0:64, :],
                            qs[0:64, :],
                            start=True,
                            stop=True,
                        )
                        nc.tensor.matmul(
                            s_ps[:, 1, :],
                            ks[64:128, :],
                            qs[64:128, :],
                            start=True,
                            stop=True,
                        )
                        e = e_pool.tile([128, 2, 512], BF16, tag="e")
                        nc.scalar.activation(
                            out=e[:],
                            in_=s_ps[:],
                            func=mybir.ActivationFunctionType.Exp,
                            bias=zero_sb[:, 0:1],
                            scale=0.125,
                        )
                        nc.tensor.matmul(
                            att0[0 : DH + 1, :],
                            v_sb[:, kt, h0, :],
                            e[:, 0, :],
                            start=(kt == 0),
                            stop=(kt == KT - 1),
                        )
                        nc.tensor.matmul(
                            att1[0 : DH + 1, :],
                            v_sb[:, kt, h1, :],
                            e[:, 1, :],
                            start=(kt == 0),
                            stop=(kt == KT - 1),
                        )
                    # normalize: rows 0:64 scaled by 1/row 64
                    for h, att in ((h0, att0), (h1, att1)):
                        # free the PSUM bank fast, then normalize from SBUF
                        atmp = tmp_pool.tile([DH + 1, 512], F32, tag="atmp")
                        nc.vector.tensor_copy(atmp[:], att[0 : DH + 1, :])
                        rec0 = rb_pool.tile([1, 512], F32, tag="rec0")
                        nc.sync.dma_start(rec0[0:1, :], atmp[DH : DH + 1, :])
                        nc.vector.reciprocal_approx_fast(rec0[0:1, :], rec0[0:1, :])
                        rb = rb_pool.tile([64, 512], F32, tag="rb")
                        nc.gpsimd.partition_broadcast(rb[:], rec0[0:1, :])
                        dst_cols = att_sb[:, hp, qb * 512 : (qb + 1) * 512]
                        if h % 2 == 0:
                            nc.vector.tensor_mul(
                                dst_cols[0:64, :], atmp[0:DH, :], rb[:]
                            )
                        else:
                            tmp_n = tmp_pool.tile([64, 512], F32R, tag="tmp")
                            nc.vector.tensor_mul(tmp_n[:], atmp[0:DH, :], rb[:])
                            nc.sync.dma_start(out=dst_cols[64:128, :], in_=tmp_n[:])

                # ---- phase C for this q-block ----
                for t in range(4):
                    qt_i = qb * 4 + t
                    y_ps = mm_ps_pool.tile([128, 2, 512], F32, tag="mm")
                    for c in range(HC):
                        lhsT = att_sb[:, c, qt_i * 128 : (qt_i + 1) * 128]
                        nc.tensor.matmul(
                            y_ps[:, 0, :],
                            lhsT,
                            wo_sb[:, c, 0:512],
                            start=(c == 0),
                            stop=(c == HC - 1),
                        )
                        nc.tensor.matmul(
                            y_ps[:, 1, 0 : VD - 512],
                            lhsT,
                            wo_sb[:, c, 512:VD],
                            start=(c == 0),
                            stop=(c == HC - 1),
                        )
                    y_sb = y_sb_pool.tile([128, VD], F32, tag="ysb")
                    nc.vector.tensor_add(y_sb[:, 0:512], y_ps[:, 0, :], bo_sb[:, 0:512])
                    nc.vector.tensor_add(
                        y_sb[:, 512:VD], y_ps[:, 1, 0 : VD - 512], bo_sb[:, 512:VD]
                    )
                    nc.sync.dma_start(
                        out=y_d.rearrange("(t p) v -> t p v", p=128)[qt_i],
                        in_=y_sb[:],
                    )


_BUILT = None


def _build():
    global _BUILT
    if _BUILT is None:
        nc = bacc.Bacc(
            "TRN2", target_bir_lowering=False, debug=False, num_devices=N_CORES
        )
        with tile.TileContext(nc) as tc:
            _kernel_body(tc)
        nc.compile()
        _BUILT = nc
    return _BUILT


def _prepare_in_maps(text_embeds, Wq, bq, Wk, bk, Wv, bv, Wo, bo):
    text_embeds = np.asarray(text_embeds, np.float32)
    Wq = np.ascontiguousarray(np.asarray(Wq, np.float32))
    Wk = np.ascontiguousarray(np.asarray(Wk, np.float32))
    Wv = np.ascontiguousarray(np.asarray(Wv, np.float32))
    Wo = np.ascontiguousarray(np.asarray(Wo, np.float32))
    bq = np.ascontiguousarray(np.asarray(bq, np.float32))
    bo2 = (
        np.asarray(bo, np.float64) + np.asarray(bv, np.float64) @ np.asarray(Wo, np.float64)
    ).astype(np.float32)
    in_maps = []
    for core in range(N_CORES):
        b, half = divmod(core, 2)
        xt = np.ascontiguousarray(text_embeds[b].T)  # [D, S]
        if half:
            xt = np.ascontiguousarray(np.roll(xt, -SQ, axis=1))
        in_maps.append(
            {
                "xt": xt,
                "wq": Wq,
                "wk": Wk,
                "wv": Wv,
                "bq": bq,
                "wo": Wo,
                "bo2": bo2,
            }
        )
    return in_maps


def _assemble(results):
    out = np.empty((B, S, VD), np.float32)
    for core in range(N_CORES):
        b, half = divmod(core, 2)
        out[b, half * SQ : (half + 1) * SQ] = results[core]["y"]
    return out


def run(trace=False, **inputs):
    nc = _build()
    in_maps = _prepare_in_maps(**inputs)
    res = bass_utils.run_bass_kernel_spmd(
        nc, in_maps, core_ids=list(range(N_CORES)), trace=trace
    )
    return _assemble(res.results), res


def kernel(**inputs):
    out, _ = run(trace=False, **inputs)
    return out
